# revision 1
# baseline (speedup 1.0000x reference)
"""Multi-head attention (B=4, S=2048, D=1024, H=16) on 8 trn2 NeuronCores.

Sharding: tensor-parallel over heads — core c owns heads [2c, 2c+1]
(= model dims [128c, 128c+128)).  Each core computes q/k/v projections for
its head slice (full batch), local attention, and a partial out-projection
against its 128 columns of Wo.  The 8 partial [B*S, D] outputs are summed
on the host (the all-reduce of the Megatron pattern, done at gather time).

Per-core kernel (bf16 matmul operands, fp32 PSUM accumulation):
  xT  [D, B*S]   : x transposed on host so the contraction dim lands on
                   SBUF partitions (avoids on-chip transposes of x).
  qT/kT [128, S] per batch : head-major [2*64, tokens].
  scores_T [k, q] in PSUM -> exp via ScalarE (scale=1/8 fused) -> bf16.
  v_ext [k-tile, 65*2]     : v natural layout (PE-transposed) with a ones
                   column per head => attn@v yields numerator + softmax
                   denominator in one pass.
  division: fast-NR reciprocal of the denom row (fp32), K=1 fp32r matmul
                   broadcast across partitions, DVE multiply.
  out-proj: outT [128, S] (head dims on partitions) @ WoT slice.

Scheduling: Tile's per-engine issue order follows emission order, and PSUM
accumulation groups MUST be contiguous on the PE (interleaving any other
matmul corrupts them / NRT_EXEC_UNIT_UNRECOVERABLE).  The attention exp is
ScalarE-paced, so scores phases leave the PE under-occupied and the HAM
clock-gate re-throttles it to 1.2 GHz.  To keep the PE dense we hand-
interleave self-contained PE work units (projection groups, v-transposes,
attn@v groups of the previous q-chunk, divisions) between the score
matmuls of the current q-chunk — each unit internally contiguous.
"""
import os
import sys

sys.path.insert(0, "/opt/trn_rl_repo")

import numpy as np

import concourse.mybir as mybir
import concourse.tile as tile
from concourse import bacc
from concourse._compat import with_exitstack
from concourse.bass_utils import run_bass_kernel_spmd
from concourse.masks import make_identity
from contextlib import ExitStack

B, S, D, H = 4, 2048, 1024, 16
HD = D // H              # 64
P = 128
NCORES = 8
NH = H // NCORES         # 2 heads per core
T = B * S                # 8192 tokens
DT = D // P              # 8 contraction tiles
KT = S // P              # 16 k-tiles per batch
QC = 1024                # q-chunk (2 psum banks, halves ACT overhead)
NQC = S // QC            # 2
HQ = 512                 # matmul free-dim chunk (one psum bank)
TC = 512                 # projection token chunk
NTC = S // TC            # 4
EXP_SCALE = float(1.0 / np.sqrt(HD))

f32 = mybir.dt.float32
f32r = mybir.dt.float32r
bf16 = mybir.dt.bfloat16

LAST_EXEC_TIME_NS = None
_CACHED_NC = None


@with_exitstack
def _mha_kernel(ctx: ExitStack, tc_: tile.TileContext, ins, outs):
    nc = tc_.nc
    xT_d, wqT_d, wkT_d, wvT_d, woT_d, ones_d = ins
    out_d = outs[0]

    const = ctx.enter_context(tc_.tile_pool(name="const", bufs=1))
    wpool = ctx.enter_context(tc_.tile_pool(name="wpool", bufs=1))
    xpool = ctx.enter_context(tc_.tile_pool(name="xpool", bufs=3))
    qpool = ctx.enter_context(tc_.tile_pool(name="qpool", bufs=1))
    kpool = ctx.enter_context(tc_.tile_pool(name="kpool", bufs=1))
    vpool = ctx.enter_context(tc_.tile_pool(name="vpool", bufs=1))
    vxpool = ctx.enter_context(tc_.tile_pool(name="vxpool", bufs=1))
    epool = ctx.enter_context(tc_.tile_pool(name="epool", bufs=48))
    opool = ctx.enter_context(tc_.tile_pool(name="opool", bufs=1))
    rpool = ctx.enter_context(tc_.tile_pool(name="rpool", bufs=2))
    ospool = ctx.enter_context(tc_.tile_pool(name="ospool", bufs=2))

    # single PSUM pool: 4 rotating [128,1024] slots (2 banks each)
    psum = ctx.enter_context(tc_.tile_pool(name="psum", bufs=4, space="PSUM"))

    ident = const.tile([P, P], f32, tag="ident")
    make_identity(nc, ident)
    ones_b = const.tile([P, 2], bf16, tag="ones_b")
    nc.gpsimd.dma_start(ones_b[:], ones_d[:, 0:2])
    ones_r = const.tile([1, HD], f32r, tag="ones_r")
    nc.sync.dma_start(ones_r[:], ones_d[0:1, 2:2 + HD].bitcast(f32r))

    wq = wpool.tile([P, D], bf16, tag="wq")
    wk = wpool.tile([P, D], bf16, tag="wk")
    wv = wpool.tile([P, D], bf16, tag="wv")
    wo = wpool.tile([P, D], bf16, tag="wo")

    xT_r = xT_d[:].rearrange("(dt p) t -> p dt t", p=P)

    xts = {}

    def load_x(t, lb):
        xt = xpool.tile([P, DT * TC], bf16, tag="xt", name=f"xt_{lb}_{t}")
        nc.gpsimd.dma_start(
            xt[:].rearrange("p (dt t) -> p dt t", dt=DT),
            xT_r[:, :, lb * S + t * TC: lb * S + (t + 1) * TC],
        )
        xts[(lb, t)] = xt

    def load_w(w_sb, w_d):
        # [D, 128] DRAM -> [128, DT*128] SBUF, d-tile major, cast to bf16
        nc.gpsimd.dma_start(
            w_sb[:].rearrange("p (dt o) -> p dt o", dt=DT),
            w_d[:].rearrange("(dt p) o -> p dt o", p=P),
        )

    # first projection (k, chunk 0) only needs wk + x0: ship those first
    load_w(wk, wkT_d)
    load_x(0, 0)
    load_w(wq, wqT_d)
    load_x(1, 0)
    load_w(wv, wvT_d)
    nc.gpsimd.dma_start(wo[:], woT_d[:])

    carry = []   # deferred out-projections of the previous batch

    for b in range(B):
        qT = qpool.tile([P, S], bf16, tag="qT")
        kT = kpool.tile([P, S], bf16, tag="kT")
        vT = vpool.tile([P, S], f32, tag="vT")
        v_ext = vxpool.tile([P, KT * 2 * (HD + 1)], bf16, tag="vext")
        outT = opool.tile([P, S], bf16, tag="outT")


        def proj(w_sb, dstT, t):
            # one contiguous 8-matmul accumulation group + eviction
            pp = psum.tile([P, QC], f32, tag="ps")
            for dt in range(DT):
                nc.tensor.matmul(
                    pp[:, 0:TC],
                    w_sb[:, dt * P:(dt + 1) * P],
                    xts[(b, t)][:, dt * TC:(dt + 1) * TC],
                    start=(dt == 0), stop=(dt == DT - 1),
                )
            nc.vector.tensor_copy(dstT[:, t * TC:(t + 1) * TC], pp[:, 0:TC])

        def vtrans(kt):
            vps = psum.tile([P, QC], f32, tag="ps")
            nc.tensor.transpose(
                vps[:, 0:P], vT[:, kt * P:(kt + 1) * P], ident[:])
            base = kt * 2 * (HD + 1)
            nc.vector.tensor_copy(v_ext[:, base:base + HD], vps[:, 0:HD])
            nc.vector.tensor_copy(
                v_ext[:, base + HD + 1:base + 2 * HD + 1], vps[:, HD:2 * HD])
            nc.vector.tensor_copy(v_ext[:, base + HD:base + HD + 1],
                                  ones_b[:, 0:1])
            nc.vector.tensor_copy(
                v_ext[:, base + 2 * HD + 1:base + 2 * HD + 2], ones_b[:, 1:2])

        def scores(qc, kt, exp_tiles):
            for h in range(NH):
                hs = slice(h * HD, (h + 1) * HD)
                sc = psum.tile([P, QC], f32, tag="ps")
                for half in range(QC // HQ):
                    nc.tensor.matmul(
                        sc[:, half * HQ:(half + 1) * HQ],
                        kT[hs, kt * P:(kt + 1) * P],
                        qT[hs, qc * QC + half * HQ: qc * QC + (half + 1) * HQ],
                        start=True, stop=True,
                    )
                ex = epool.tile([P, QC], bf16, tag="exp")
                nc.scalar.activation(
                    ex[:], sc[:], mybir.ActivationFunctionType.Exp,
                    scale=EXP_SCALE)
                exp_tiles[h][kt] = ex

        def attnv_group(oes, h, half, exp_tiles):
            # one contiguous 16-matmul accumulation group
            if oes[h] is None:
                oes[h] = psum.tile([P, QC], f32, tag="ps", name=f"oe_h{h}")
            hq = slice(half * HQ, (half + 1) * HQ)
            for kt in range(KT):
                base = kt * 2 * (HD + 1) + h * (HD + 1)
                nc.tensor.matmul(
                    oes[h][0:HD + 1, hq],
                    v_ext[:, base:base + HD + 1],
                    exp_tiles[h][kt][:, hq],
                    start=(kt == 0), stop=(kt == KT - 1),
                )

        def div_pre(oes, h, recrs):
            # DVE-only reciprocal chain; emit as early as possible
            oe = oes[h]
            den = rpool.tile([1, QC], f32, tag="den")
            # custom-DVE ops mis-read partition-offset inputs: stage the
            # denominator row to partition 0 first
            nc.vector.tensor_copy(den[:], oe[HD:HD + 1, :])
            rec = rpool.tile([1, QC], f32, tag="rec")
            scr = rpool.tile([1, QC], f32, tag="scr")
            nc.vector.reciprocal_approx_accurate(rec[:], den[:], scr[:])
            recr = rpool.tile([1, QC], f32r, tag="recr")
            with nc.allow_low_precision(reason="feeds bcast matmul"):
                nc.vector.tensor_copy(recr[:], rec[:])
            recrs[h] = recr

        def div_post(oes, qc, h, recrs):
            oe = oes[h]
            qs = slice(qc * QC, (qc + 1) * QC)
            recr = recrs[h]
            rb_ps = psum.tile([P, QC], f32, tag="ps")
            for half in range(QC // HQ):
                hq = slice(half * HQ, (half + 1) * HQ)
                nc.tensor.matmul(rb_ps[0:HD, hq], ones_r[:], recr[:, hq],
                                 start=True, stop=True)
            rb = rpool.tile([HD, QC], f32, tag="rb")
            nc.vector.tensor_copy(rb[:], rb_ps[0:HD, :])
            nc.vector.tensor_mul(outT[h * HD:(h + 1) * HD, qs],
                                 oe[0:HD, :], rb[:])

        def oproj(t, oT=outT, bb=b):
            # oT/bb bound at def time: carried closures must keep THIS
            # batch's outT and row base, not the next iteration's
            po = psum.tile([P, QC], f32, tag="ps")
            for ec in range(D // HQ):
                nc.tensor.matmul(
                    po[:, ec * HQ:(ec + 1) * HQ],
                    oT[:, t * P:(t + 1) * P],
                    wo[:, ec * HQ:(ec + 1) * HQ],
                    start=True, stop=True,
                )
            osb = ospool.tile([P, D], f32, tag="osb")
            nc.vector.tensor_copy(osb[:], po[:])
            nc.sync.dma_start(
                out_d[bb * S + t * P: bb * S + (t + 1) * P, :],
                osb[:],
            )

        # ---------- batch schedule ----------
        # prologue: x chunks 0-1, k/q/v projections 0-1, v-transposes 0-7,
        # interleaved with the previous batch's deferred out-projections
        if (b, 0) not in xts:
            load_x(0, b)
        if (b, 1) not in xts:
            load_x(1, b)
        prologue = []
        for t in (0, 1):
            prologue += [
                (lambda tt: lambda: proj(wk, kT, tt))(t),
                (lambda tt: lambda: proj(wq, qT, tt))(t),
                (lambda tt: lambda: proj(wv, vT, tt))(t),
            ]
        prologue.append(lambda: load_x(2, b))
        prologue.append(lambda: (vtrans(0), vtrans(1), vtrans(2), vtrans(3)))
        prologue.append(lambda: (vtrans(4), vtrans(5), vtrans(6), vtrans(7)))
        for u in prologue:
            u()
            if carry:
                carry.pop(0)()
        while carry:
            carry.pop(0)()

        exp0 = [[None] * KT for _ in range(NH)]
        # qc0 scores with the remaining projections/transposes as PE filler
        fillers = [
            lambda: proj(wk, kT, 2),
            lambda: (load_x(3, b), proj(wk, kT, 3)),
            lambda: proj(wq, qT, 2),
            lambda: proj(wv, vT, 2),
            lambda: (vtrans(8), vtrans(9)),
            lambda: proj(wq, qT, 3),
            lambda: proj(wv, vT, 3),
            lambda: (vtrans(10), vtrans(11)),
            lambda: (vtrans(12), vtrans(13)),
            lambda: (vtrans(14), vtrans(15)),
        ]
        n0 = len(fillers)
        done0 = 0
        for kt in range(KT):
            scores(0, kt, exp0)
            want = (kt + 1) * n0 // KT
            while done0 < want and fillers:
                fillers.pop(0)()
                done0 += 1

        # qc1 scores with qc0's attn@v groups + divisions as PE filler
        exp1 = [[None] * KT for _ in range(NH)]
        oes0 = [None, None]
        recrs0 = [None, None]
        fillers = [
            lambda: attnv_group(oes0, 0, 0, exp0),
            lambda: attnv_group(oes0, 0, 1, exp0),
            lambda: (div_pre(oes0, 0, recrs0), attnv_group(oes0, 1, 0, exp0)),
            lambda: div_post(oes0, 0, 0, recrs0),
            lambda: (attnv_group(oes0, 1, 1, exp0),
                     div_pre(oes0, 1, recrs0),
                     load_x(0, b + 1) if b + 1 < B else None),
            lambda: (div_post(oes0, 0, 1, recrs0),
                     load_x(1, b + 1) if b + 1 < B else None),
        ] + [(lambda tt: lambda: oproj(tt))(t_) for t_ in range(6)]
        n1 = len(fillers)
        done1 = 0
        for kt in range(KT):
            scores(1, kt, exp1)
            want = (kt + 1) * n1 // KT
            while done1 < want and fillers:
                fillers.pop(0)()
                done1 += 1
        for f in fillers:
            f()

        # flush qc1 attn@v + divisions, interleaved with out-projection
        oes1 = [None, None]
        recrs1 = [None, None]
        oproj(6)
        attnv_group(oes1, 0, 0, exp1)
        oproj(7)
        attnv_group(oes1, 0, 1, exp1)
        div_pre(oes1, 0, recrs1)
        attnv_group(oes1, 1, 0, exp1)
        div_post(oes1, 1, 0, recrs1)
        attnv_group(oes1, 1, 1, exp1)
        div_pre(oes1, 1, recrs1)
        div_post(oes1, 1, 1, recrs1)
        carry = [(lambda tt, op: lambda: op(tt))(t_, oproj)
                 for t_ in range(8, S // P)]
        if b == B - 1:
            while carry:
                carry.pop(0)()


def _build():
    global _CACHED_NC
    if _CACHED_NC is not None:
        return _CACHED_NC
    nc = bacc.Bacc("TRN2", target_bir_lowering=False, debug=False)
    xT = nc.dram_tensor("xT", [D, T], f32, kind="ExternalInput").ap()
    wqT = nc.dram_tensor("wqT", [D, P], f32, kind="ExternalInput").ap()
    wkT = nc.dram_tensor("wkT", [D, P], f32, kind="ExternalInput").ap()
    wvT = nc.dram_tensor("wvT", [D, P], f32, kind="ExternalInput").ap()
    woT = nc.dram_tensor("woT", [P, D], f32, kind="ExternalInput").ap()
    ones = nc.dram_tensor("ones", [P, HD + 2], f32, kind="ExternalInput").ap()
    out = nc.dram_tensor("out", [T, D], f32, kind="ExternalOutput").ap()

    with tile.TileContext(nc) as tc_:
        _mha_kernel(tc_, [xT, wqT, wkT, wvT, woT, ones], [out])
    nc.compile()
    _CACHED_NC = nc
    return nc


def kernel(x: np.ndarray, Wq: np.ndarray, Wk: np.ndarray, Wv: np.ndarray,
           Wo: np.ndarray) -> np.ndarray:
    global LAST_EXEC_TIME_NS
    nc = _build()

    x = np.asarray(x, dtype=np.float32)
    xT = np.ascontiguousarray(x.reshape(T, D).T)          # [D, T]
    ones_in = np.ones((P, HD + 2), dtype=np.float32)

    in_maps = []
    for c in range(NCORES):
        rows = slice(c * P, (c + 1) * P)
        in_maps.append({
            "xT": xT,
            "wqT": np.ascontiguousarray(np.asarray(Wq, np.float32)[rows, :].T),
            "wkT": np.ascontiguousarray(np.asarray(Wk, np.float32)[rows, :].T),
            "wvT": np.ascontiguousarray(np.asarray(Wv, np.float32)[rows, :].T),
            "woT": np.ascontiguousarray(np.asarray(Wo, np.float32)[:, rows].T),
            "ones": ones_in,
        })

    trace = bool(os.environ.get("BASS_TRACE"))
    res = run_bass_kernel_spmd(nc, in_maps, core_ids=list(range(NCORES)),
                               trace=trace)
    LAST_EXEC_TIME_NS = res.exec_time_ns

    acc = res.results[0]["out"].astype(np.float32)
    for c in range(1, NCORES):
        acc = acc + res.results[c]["out"]
    return acc.reshape(B, S, D)



# revision 14
# speedup vs baseline: 1.0584x; 1.0584x over previous
"""Multi-head attention (B=4, S=2048, D=1024, H=16) on 8 trn2 NeuronCores.

Sharding: tensor-parallel over heads — core c owns heads [2c, 2c+1]
(= model dims [128c, 128c+128)).  Each core computes q/k/v projections for
its head slice (full batch), local attention, and a partial out-projection
against its 128 columns of Wo.  The 8 partial [B*S, D] outputs (bf16) are
summed on the host (the all-reduce of the Megatron pattern, at gather time).

Per-core kernel (bf16 matmul operands, fp32 PSUM accumulation):
  xT  [D, B*S]   : x transposed on host so the contraction dim lands on
                   SBUF partitions (avoids on-chip transposes of x).
  qT/kT [128, S] per batch : head-major [2*64, tokens].
  scores_T [k, q] in PSUM -> exp via ScalarE (scale=1/8 fused) -> bf16.
  v_ext [k-tile, 2*(64+1)]: v natural layout (PE-transposed) with a ones
                   column per head => attn@v yields numerator (rows 0-63) +
                   softmax denominator (row 64) in one pass.
  division: denom row staged to partition 0, fast-NR reciprocal (fp32),
                   K=1 f32r matmul broadcast across partitions, DVE multiply.
  out-proj: outT [128, S] (head dims on partitions) @ WoT slice -> bf16 out.

Scheduling: the kernel is ScalarE-bound (33.5M exp elements/core ~ 295us of
ACT time); the PE total is ~285us.  Both must run >90% utilized, so the
schedule is one continuous software pipeline across batches: a kt-stream of
score matmul pairs (two heads run concurrently on disjoint PE row groups via
auto tile_position) + per-head exp ACTs, with ALL other PE work (attn@v of
the previous q-chunk, divisions, out-projection of the previous batch,
q/k/v projections of the NEXT batch, v transposes) emitted between score
pairs by a deficit-paced work queue with per-unit deadlines.  PSUM: 2x
2-bank score slots, 2x 1-bank attn@v accumulators, 2x 1-bank filler slots.
PSUM accumulation groups stay contiguous on the PE (interleaving another
matmul into a group corrupts it).
"""
import os
import sys

sys.path.insert(0, "/opt/trn_rl_repo")

import numpy as np

import concourse.mybir as mybir
import concourse.tile as tile
from concourse import bacc
from concourse._compat import with_exitstack
from concourse.bass_utils import run_bass_kernel_spmd
from concourse.masks import make_identity
from contextlib import ExitStack

B, S, D, H = 4, 2048, 1024, 16
HD = D // H              # 64
P = 128
NCORES = 8
NH = H // NCORES         # 2 heads per core
T = B * S                # 8192 tokens
DT = D // P              # 8 contraction tiles
KT = S // P              # 16 k-tiles per batch
QC = 1024                # q-chunk (2 psum banks)
NQC = S // QC            # 2
HQ = 512                 # matmul free-dim chunk (one psum bank)
TC = 512                 # projection token chunk
NTC = S // TC            # 4
VB = 2 * (HD + 1)        # v_ext block per k-tile: [1|v0|1|v1]
EXP_SCALE = float(1.0 / np.sqrt(HD))

f32 = mybir.dt.float32
f32r = mybir.dt.float32r
bf16 = mybir.dt.bfloat16

LAST_EXEC_TIME_NS = None
_CACHED_NC = None


@with_exitstack
def _mha_kernel(ctx: ExitStack, tc_: tile.TileContext, ins, outs):
    nc = tc_.nc
    xT_d, wqT_d, wkT_d, wvT_d, woT_d, ones_d = ins
    out_d = outs[0]

    const = ctx.enter_context(tc_.tile_pool(name="const", bufs=1))
    wpool = ctx.enter_context(tc_.tile_pool(name="wpool", bufs=1))
    xpool = ctx.enter_context(tc_.tile_pool(name="xpool", bufs=3))
    qpool = ctx.enter_context(tc_.tile_pool(name="qpool", bufs=2))
    kpool = ctx.enter_context(tc_.tile_pool(name="kpool", bufs=2))
    vpool = ctx.enter_context(tc_.tile_pool(name="vpool", bufs=2))
    vxpool = ctx.enter_context(tc_.tile_pool(name="vxpool", bufs=2))
    epool = ctx.enter_context(tc_.tile_pool(name="epool", bufs=52))
    opool = ctx.enter_context(tc_.tile_pool(name="opool", bufs=2))
    rpool = ctx.enter_context(tc_.tile_pool(name="rpool", bufs=2))
    ospool = ctx.enter_context(tc_.tile_pool(name="ospool", bufs=2))

    # PSUM: 2x scores slots (2 banks each) + 2x attnv accumulators (1 bank)
    # + 2x filler slots (1 bank) = 8 banks
    spsum = ctx.enter_context(tc_.tile_pool(name="spsum", bufs=2, space="PSUM"))
    apsum = ctx.enter_context(tc_.tile_pool(name="apsum", bufs=2, space="PSUM"))
    fpsum = ctx.enter_context(tc_.tile_pool(name="fpsum", bufs=2, space="PSUM"))

    ident = const.tile([P, P], f32, tag="ident")
    make_identity(nc, ident)
    ident_b = const.tile([P, P], bf16, tag="ident_b")
    nc.vector.tensor_copy(ident_b[:], ident[:])
    ones32 = const.tile([P, 2 * KT], bf16, tag="ones32")
    nc.gpsimd.dma_start(ones32[:], ones_d[:, 0:2 * KT])
    ones_r = const.tile([1, HD], f32r, tag="ones_r")
    nc.sync.dma_start(ones_r[:], ones_d[0:1, 2 * KT:2 * KT + HD].bitcast(f32r))

    wq = wpool.tile([P, D], bf16, tag="wq")
    wk = wpool.tile([P, D], bf16, tag="wk")
    wv = wpool.tile([P, D], bf16, tag="wv")
    wo = wpool.tile([P, D], bf16, tag="wo")

    xT_r = xT_d[:].rearrange("(dt p) t -> p dt t", p=P)

    xts = {}

    def load_x(b, t):
        if (b, t) in xts or b >= B:
            return
        xt = xpool.tile([P, DT * TC], bf16, tag="xt", name=f"xt_{b}_{t}")
        nc.gpsimd.dma_start(
            xt[:].rearrange("p (dt t) -> p dt t", dt=DT),
            xT_r[:, :, b * S + t * TC: b * S + (t + 1) * TC],
        )
        xts[(b, t)] = xt

    def load_w(w_sb, w_d):
        # [D, 128] DRAM -> [128, DT*128] SBUF, d-tile major, cast to bf16
        nc.gpsimd.dma_start(
            w_sb[:].rearrange("p (dt o) -> p dt o", dt=DT),
            w_d[:].rearrange("(dt p) o -> p dt o", p=P),
        )

    load_w(wk, wkT_d)
    load_x(0, 0)
    load_w(wq, wqT_d)
    load_x(0, 1)
    load_w(wv, wvT_d)
    nc.gpsimd.dma_start(wo[:], woT_d[:])

    # ---- per-batch tile registries (filled as the pipeline advances) ----
    qTs, kTs, vTs, vxs, outTs = {}, {}, {}, {}, {}
    exps = {}    # (b, qc, h, kt) -> exp tile [P, QC] bf16
    oes = {}     # (b, qc, h, half) -> attnv psum accumulator

    def get_batch_tiles(b):
        if b not in qTs:
            qTs[b] = qpool.tile([P, S], bf16, tag="qT", name=f"qT{b}")
            kTs[b] = kpool.tile([P, S], bf16, tag="kT", name=f"kT{b}")
            vTs[b] = vpool.tile([P, S], bf16, tag="vT", name=f"vT{b}")

    def proj(w_sb, dst_of, b, t):
        # one contiguous 8-matmul accumulation group + bf16 eviction
        get_batch_tiles(b)
        load_x(b, t + 2) if t + 2 < NTC else load_x(b + 1, t + 2 - NTC)
        pp = fpsum.tile([P, HQ], f32, tag="fp")
        for dt in range(DT):
            nc.tensor.matmul(
                pp[:],
                w_sb[:, dt * P:(dt + 1) * P],
                xts[(b, t)][:, dt * TC:(dt + 1) * TC],
                start=(dt == 0), stop=(dt == DT - 1),
            )
        nc.vector.tensor_copy(dst_of(b)[:, t * TC:(t + 1) * TC], pp[:])

    def get_vext(b):
        if b not in vxs:
            vx = vxpool.tile([P, KT * VB], bf16, tag="vext", name=f"vx{b}")
            # prewrite the ones columns (cols kt*VB + {64, 129}) once
            nc.vector.tensor_copy(
                vx[:].rearrange("p (kt ho c) -> p kt ho c", kt=KT, ho=2)
                    [:, :, :, HD:HD + 1],
                ones32[:].rearrange("p (kt ho c) -> p kt ho c", kt=KT, ho=2),
            )
            vxs[b] = vx
        return vxs[b]

    def vtrans(b, kt):
        vx = get_vext(b)
        vps = fpsum.tile([P, P], bf16, tag="fp")
        nc.tensor.transpose(vps[:], vTs[b][:, kt * P:(kt + 1) * P],
                            ident_b[:])
        base = kt * VB
        nc.vector.tensor_copy(vx[:, base:base + HD], vps[:, 0:HD])
        nc.vector.tensor_copy(vx[:, base + HD + 1:base + 2 * HD + 1],
                              vps[:, HD:2 * HD])

    def attnv(b, qc, h, half):
        # one contiguous 16-matmul accumulation group
        oe = apsum.tile([P, HQ], f32, tag="ap", name=f"oe_{b}_{qc}_{h}_{half}")
        oes[(b, qc, h, half)] = oe
        vx = vxs[b]
        hq = slice(half * HQ, (half + 1) * HQ)
        for kt in range(KT):
            base = kt * VB + h * (HD + 1)
            nc.tensor.matmul(
                oe[0:HD + 1, :],
                vx[:, base:base + HD + 1],
                exps[(b, qc, h, kt)][:, hq],
                start=(kt == 0), stop=(kt == KT - 1),
            )

    def get_outT(b):
        if b not in outTs:
            outTs[b] = opool.tile([P, S], bf16, tag="outT", name=f"outT{b}")
        return outTs[b]

    def div(b, qc, h, half):
        oe = oes.pop((b, qc, h, half))
        # custom-DVE ops mis-read partition-offset inputs: stage the
        # denominator row to partition 0 first
        den = rpool.tile([1, HQ], f32, tag="den")
        nc.vector.tensor_copy(den[:], oe[HD:HD + 1, :])
        rec = rpool.tile([1, HQ], f32, tag="rec")
        scr = rpool.tile([1, HQ], f32, tag="scr")
        nc.vector.reciprocal_approx_accurate(rec[:], den[:], scr[:])
        recr = rpool.tile([1, HQ], f32r, tag="recr")
        with nc.allow_low_precision(reason="feeds bcast matmul"):
            nc.vector.tensor_copy(recr[:], rec[:])
        rb_ps = fpsum.tile([P, HQ], f32, tag="fp")
        nc.tensor.matmul(rb_ps[0:HD, :], ones_r[:], recr[:],
                         start=True, stop=True)
        rb = rpool.tile([HD, HQ], f32, tag="rb")
        nc.vector.tensor_copy(rb[:], rb_ps[0:HD, :])
        qs = slice(qc * QC + half * HQ, qc * QC + (half + 1) * HQ)
        nc.vector.tensor_mul(get_outT(b)[h * HD:(h + 1) * HD, qs],
                             oe[0:HD, :], rb[:])

    def oproj(b, t):
        oT = outTs[b]
        osb = ospool.tile([P, D], bf16, tag="osb")
        for ec in range(D // HQ):
            po = fpsum.tile([P, HQ], f32, tag="fp")
            nc.tensor.matmul(
                po[:],
                oT[:, t * P:(t + 1) * P],
                wo[:, ec * HQ:(ec + 1) * HQ],
                start=True, stop=True,
            )
            nc.vector.tensor_copy(osb[:, ec * HQ:(ec + 1) * HQ], po[:])
        nc.sync.dma_start(
            out_d[b * S + t * P: b * S + (t + 1) * P, :],
            osb[:],
        )

    # ---------- work queue with deficit pacing + deadlines ----------
    queue = []   # list of [cost_cycles, deadline_slot, fn]
    state = {"deficit": 0.0}

    def push(cost, deadline, fn):
        queue.append([cost, deadline, fn])

    def pump(slot, allot):
        # force-emit past-deadline units (and, since queue order encodes
        # dependencies, everything queued before them), then paced emission
        state["deficit"] += allot
        last = -1
        for i, u in enumerate(queue):
            if u[1] <= slot:
                last = i
        for _ in range(last + 1):
            cost, _, fn = queue.pop(0)
            fn()
            state["deficit"] -= cost
        while queue and state["deficit"] > 0:
            cost, _, fn = queue.pop(0)
            fn()
            state["deficit"] -= cost

    NEVER = 10 ** 9
    C_PROJ, C_ATTNV, C_OPROJ, C_VT, C_DIV = 4096, 8192, 1024, 130, 512

    def slot_of(b, qc, kt):
        return (b * NQC + qc) * KT + kt

    def push_phase_supply(b, qc):
        """Queue the filler work for phase (b, qc), in dependency order."""
        base = slot_of(b, qc, 0)
        if qc == 0:
            pb, pqc = b - 1, 1
        else:
            pb, pqc = b, 0
        # attn@v + divisions of the previous phase's exps, then the
        # out-projections they unblock (qc0 -> t 0..7, qc1 -> t 8..15)
        if pb >= 0:
            t0 = 0 if pqc == 0 else 8
            push(C_ATTNV, NEVER, lambda: attnv(pb, pqc, 0, 0))
            push(C_ATTNV, NEVER, lambda: attnv(pb, pqc, 0, 1))
            push(C_DIV, NEVER, lambda: div(pb, pqc, 0, 0))
            push(C_ATTNV, NEVER, lambda: attnv(pb, pqc, 1, 0))
            push(C_DIV, NEVER, lambda: div(pb, pqc, 0, 1))
            push(C_ATTNV, NEVER, lambda: attnv(pb, pqc, 1, 1))
            push(C_DIV, NEVER, lambda: div(pb, pqc, 1, 0))
            for t_ in (t0, t0 + 1):
                push(C_OPROJ, NEVER, (lambda tt: lambda: oproj(pb, tt))(t_))
            push(C_DIV, NEVER, lambda: div(pb, pqc, 1, 1))
            for t_ in range(t0 + 2, t0 + 8):
                push(C_OPROJ, NEVER, (lambda tt: lambda: oproj(pb, tt))(t_))
        # next batch's projections + v transposes, split across the two
        # phases; deadlines pin them before their consumers.
        nb = b + 1
        if nb < B:
            if qc == 0:
                # k0,q0,v0,k1,q1,v1 — k/q needed by scores(nb, 0) start
                dl_k = slot_of(nb, 0, 0)
                for t_ in (0, 1):
                    push(C_PROJ, dl_k - 14 + 4 * t_,
                         (lambda tt: lambda: proj(wk, kTs.get, nb, tt))(t_))
                    push(C_PROJ, dl_k - 12 + 4 * t_,
                         (lambda tt: lambda: proj(wq, qTs.get, nb, tt))(t_))
                    push(C_PROJ, dl_k - 10 + 4 * t_,
                         (lambda tt: lambda: proj(wv, vTs.get, nb, tt))(t_))
            else:
                dl_k = slot_of(nb, 0, 0)
                for t_ in (2, 3):
                    push(C_PROJ, dl_k - 6 + 2 * (t_ - 2),
                         (lambda tt: lambda: proj(wk, kTs.get, nb, tt))(t_))
                    push(C_PROJ, dl_k - 5 + 2 * (t_ - 2),
                         (lambda tt: lambda: proj(wq, qTs.get, nb, tt))(t_))
                    push(C_PROJ, dl_k - 4 + 2 * (t_ - 2),
                         (lambda tt: lambda: proj(wv, vTs.get, nb, tt))(t_))
                # v transposes of batch nb: needed by attnv(nb, 0, ...)
                # which is pushed at phase (nb, 1); deadline end of (nb, 0)
                dl_vt = slot_of(nb, 0, 15)
                for kt0 in range(0, KT, 4):
                    def vt_bundle(bb, k0):
                        def f():
                            for k_ in range(k0, k0 + 4):
                                vtrans(bb, k_)
                        return f
                    push(4 * C_VT, dl_vt, vt_bundle(nb, kt0))

    def scores_kt(b, qc, kt):
        ps = {}
        for h in range(NH):
            ps[h] = spsum.tile([P, QC], f32, tag="sp",
                               name=f"sc_{b}_{qc}_{kt}_{h}")
        for half in range(NQC):
            hq = slice(half * HQ, (half + 1) * HQ)
            for h in range(NH):
                hs = slice(h * HD, (h + 1) * HD)
                nc.tensor.matmul(
                    ps[h][:, hq],
                    kTs[b][hs, kt * P:(kt + 1) * P],
                    qTs[b][hs, qc * QC + half * HQ:
                           qc * QC + (half + 1) * HQ],
                    start=True, stop=True,
                )
        for h in range(NH):
            ex = epool.tile([P, QC], bf16, tag="exp",
                            name=f"ex_{b}_{qc}_{h}_{kt}")
            nc.scalar.activation(ex[:], ps[h][:],
                                 mybir.ActivationFunctionType.Exp,
                                 scale=EXP_SCALE)
            exps[(b, qc, h, kt)] = ex

    # ---------- prologue: first projections for batch 0 ----------
    load_x(0, 2)
    proj(wk, kTs.get, 0, 0)
    proj(wq, qTs.get, 0, 0)
    proj(wq, qTs.get, 0, 1)

    # batch-0 remaining projections + v transposes, deadline-gated
    push(C_PROJ, slot_of(0, 0, 2), lambda: proj(wk, kTs.get, 0, 1))
    push(C_PROJ, slot_of(0, 0, 6), lambda: proj(wk, kTs.get, 0, 2))
    push(C_PROJ, slot_of(0, 0, 7), lambda: proj(wv, vTs.get, 0, 0))
    push(C_PROJ, slot_of(0, 0, 10), lambda: proj(wk, kTs.get, 0, 3))
    push(C_PROJ, slot_of(0, 0, 12), lambda: proj(wq, qTs.get, 0, 2))
    push(C_PROJ, slot_of(0, 0, 13), lambda: proj(wq, qTs.get, 0, 3))
    push(C_PROJ, slot_of(0, 0, 14), lambda: proj(wv, vTs.get, 0, 1))
    push(C_PROJ, slot_of(0, 1, 2), lambda: proj(wv, vTs.get, 0, 2))
    push(C_PROJ, slot_of(0, 1, 4), lambda: proj(wv, vTs.get, 0, 3))
    dl_vt0 = slot_of(0, 1, 15)
    for kt0 in range(0, KT, 4):
        def vt_bundle0(k0):
            def f():
                for k_ in range(k0, k0 + 4):
                    vtrans(0, k_)
            return f
        push(4 * C_VT, dl_vt0 - (3 - kt0 // 4), vt_bundle0(kt0))

    # ---------- main pipeline ----------
    total_slots = B * NQC * KT
    for b in range(B):
        for qc in range(NQC):
            push_phase_supply(b, qc)
            # phase allotment: spread queued cost over the 16 kt slots,
            # but cap so ScalarE stays the pacer (~5500 cyc/kt of ACT).
            pending = sum(u[0] for u in queue)
            allot = min(pending / KT, 5200.0)
            for kt in range(KT):
                scores_kt(b, qc, kt)
                pump(slot_of(b, qc, kt) + 1, allot)

    # ---------- epilogue: flush everything left ----------
    push_phase_supply(B, 0)   # attnv/div/oproj for (3, qc1)
    pump(NEVER + 1, 0)


def _build():
    global _CACHED_NC
    if _CACHED_NC is not None:
        return _CACHED_NC
    nc = bacc.Bacc("TRN2", target_bir_lowering=False, debug=False)
    xT = nc.dram_tensor("xT", [D, T], f32, kind="ExternalInput").ap()
    wqT = nc.dram_tensor("wqT", [D, P], f32, kind="ExternalInput").ap()
    wkT = nc.dram_tensor("wkT", [D, P], f32, kind="ExternalInput").ap()
    wvT = nc.dram_tensor("wvT", [D, P], f32, kind="ExternalInput").ap()
    woT = nc.dram_tensor("woT", [P, D], f32, kind="ExternalInput").ap()
    ones = nc.dram_tensor("ones", [P, 2 * KT + HD], f32,
                          kind="ExternalInput").ap()
    out = nc.dram_tensor("out", [T, D], bf16, kind="ExternalOutput").ap()

    with tile.TileContext(nc) as tc_:
        _mha_kernel(tc_, [xT, wqT, wkT, wvT, woT, ones], [out])
    nc.compile()
    _CACHED_NC = nc
    return nc


def kernel(x: np.ndarray, Wq: np.ndarray, Wk: np.ndarray, Wv: np.ndarray,
           Wo: np.ndarray) -> np.ndarray:
    global LAST_EXEC_TIME_NS
    nc = _build()

    x = np.asarray(x, dtype=np.float32)
    xT = np.ascontiguousarray(x.reshape(T, D).T)          # [D, T]
    ones_in = np.ones((P, 2 * KT + HD), dtype=np.float32)

    in_maps = []
    for c in range(NCORES):
        rows = slice(c * P, (c + 1) * P)
        in_maps.append({
            "xT": xT,
            "wqT": np.ascontiguousarray(np.asarray(Wq, np.float32)[rows, :].T),
            "wkT": np.ascontiguousarray(np.asarray(Wk, np.float32)[rows, :].T),
            "wvT": np.ascontiguousarray(np.asarray(Wv, np.float32)[rows, :].T),
            "woT": np.ascontiguousarray(np.asarray(Wo, np.float32)[:, rows].T),
            "ones": ones_in,
        })

    trace = bool(os.environ.get("BASS_TRACE"))
    res = run_bass_kernel_spmd(nc, in_maps, core_ids=list(range(NCORES)),
                               trace=trace)
    LAST_EXEC_TIME_NS = res.exec_time_ns

    acc = res.results[0]["out"].astype(np.float32)
    for c in range(1, NCORES):
        acc = acc + res.results[c]["out"].astype(np.float32)
    return acc.reshape(B, S, D)


# revision 18
# speedup vs baseline: 1.1036x; 1.0427x over previous
"""Multi-head attention (B=4, S=2048, D=1024, H=16) on 8 trn2 NeuronCores.

Sharding: tensor-parallel over heads — core c owns heads [2c, 2c+1]
(= model dims [128c, 128c+128)).  Each core computes q/k/v projections for
its head slice (full batch), local attention, and a partial out-projection
against its 128 columns of Wo.  The 8 partial [B*S, D] outputs (bf16) are
summed on the host (the all-reduce of the Megatron pattern, at gather time).

Per-core kernel (bf16 matmul operands, fp32 PSUM accumulation):
  xT  [D, B*S]   : x transposed on host so the contraction dim lands on
                   SBUF partitions (avoids on-chip transposes of x).
  qT/kT [128, S] per batch : head-major [2*64, tokens].
  scores_T [k, q] in PSUM -> exp via ScalarE (scale=1/8 fused) -> bf16.
  v_ext [k-tile, 2*(64+1)]: v natural layout (PE-transposed) with a ones
                   column per head => attn@v yields numerator (rows 0-63) +
                   softmax denominator (row 64) in one pass.
  division: denom row staged to partition 0, fast-NR reciprocal (fp32),
                   K=1 f32r matmul broadcast across partitions, DVE multiply.
  out-proj: outT [128, S] (head dims on partitions) @ WoT slice -> bf16 out.

Scheduling: the kernel is ScalarE-bound (33.5M exp elements/core ~ 295us of
ACT time); the PE total is ~285us.  Both must run >90% utilized, so the
schedule is one continuous software pipeline across batches: a kt-stream of
score matmul pairs (two heads run concurrently on disjoint PE row groups via
auto tile_position) + per-head exp ACTs, with ALL other PE work (attn@v of
the previous q-chunk, divisions, out-projection of the previous batch,
q/k/v projections of the NEXT batch, v transposes) emitted between score
pairs by a deficit-paced work queue with per-unit deadlines.  PSUM: 2x
2-bank score slots, 2x 1-bank attn@v accumulators, 2x 1-bank filler slots.
PSUM accumulation groups stay contiguous on the PE (interleaving another
matmul into a group corrupts it).
"""
import os
import sys

sys.path.insert(0, "/opt/trn_rl_repo")

import numpy as np

import concourse.mybir as mybir
import concourse.tile as tile
from concourse import bacc
from concourse._compat import with_exitstack
from concourse.bass_utils import run_bass_kernel_spmd
from concourse.masks import make_identity
from contextlib import ExitStack

B, S, D, H = 4, 2048, 1024, 16
HD = D // H              # 64
P = 128
NCORES = 8
NH = H // NCORES         # 2 heads per core
T = B * S                # 8192 tokens
DT = D // P              # 8 contraction tiles
KT = S // P              # 16 k-tiles per batch
QC = 1024                # q-chunk (2 psum banks)
NQC = S // QC            # 2
HQ = 512                 # matmul free-dim chunk (one psum bank)
TC = 512                 # projection token chunk
NTC = S // TC            # 4
VB = 2 * (HD + 1)        # v_ext block per k-tile: [1|v0|1|v1]
EXP_SCALE = float(1.0 / np.sqrt(HD))

f32 = mybir.dt.float32
f32r = mybir.dt.float32r
bf16 = mybir.dt.bfloat16

LAST_EXEC_TIME_NS = None
_CACHED_NC = None


@with_exitstack
def _mha_kernel(ctx: ExitStack, tc_: tile.TileContext, ins, outs):
    nc = tc_.nc
    xT_d, wqT_d, wkT_d, wvT_d, woT_d, ones_d = ins
    out_d = outs[0]

    const = ctx.enter_context(tc_.tile_pool(name="const", bufs=1))
    wpool = ctx.enter_context(tc_.tile_pool(name="wpool", bufs=1))
    xpool = ctx.enter_context(tc_.tile_pool(name="xpool", bufs=3))
    qpool = ctx.enter_context(tc_.tile_pool(name="qpool", bufs=2))
    kpool = ctx.enter_context(tc_.tile_pool(name="kpool", bufs=2))
    vpool = ctx.enter_context(tc_.tile_pool(name="vpool", bufs=2))
    vxpool = ctx.enter_context(tc_.tile_pool(name="vxpool", bufs=2))
    epool = ctx.enter_context(tc_.tile_pool(name="epool", bufs=52))
    opool = ctx.enter_context(tc_.tile_pool(name="opool", bufs=2))
    rpool = ctx.enter_context(tc_.tile_pool(name="rpool", bufs=2))
    ospool = ctx.enter_context(tc_.tile_pool(name="ospool", bufs=2))

    # PSUM: 3x scores slots (2 banks each; 3-deep so BOTH heads' slots are
    # WAR-free when the PE reaches them -> the h0/h64 matmuls issue
    # adjacently and run concurrently on disjoint row groups) + 1 attnv
    # accumulator bank + 1 filler bank = 8 banks
    spsum = ctx.enter_context(tc_.tile_pool(name="spsum", bufs=3, space="PSUM"))
    apsum = ctx.enter_context(tc_.tile_pool(name="apsum", bufs=1, space="PSUM"))
    fpsum = ctx.enter_context(tc_.tile_pool(name="fpsum", bufs=1, space="PSUM"))

    ident = const.tile([P, P], f32, tag="ident")
    make_identity(nc, ident)
    ident_b = const.tile([P, P], bf16, tag="ident_b")
    nc.vector.tensor_copy(ident_b[:], ident[:])
    ones32 = const.tile([P, 2 * KT], bf16, tag="ones32")
    nc.gpsimd.dma_start(ones32[:], ones_d[:, 0:2 * KT])
    ones_r = const.tile([1, HD], f32r, tag="ones_r")
    nc.sync.dma_start(ones_r[:], ones_d[0:1, 2 * KT:2 * KT + HD].bitcast(f32r))

    wq = wpool.tile([P, D], bf16, tag="wq")
    wk = wpool.tile([P, D], bf16, tag="wk")
    wv = wpool.tile([P, D], bf16, tag="wv")
    wo = wpool.tile([P, D], bf16, tag="wo")

    xT_r = xT_d[:].rearrange("(dt p) t -> p dt t", p=P)

    xts = {}

    def load_x(b, t):
        if (b, t) in xts or b >= B:
            return
        xt = xpool.tile([P, DT * TC], bf16, tag="xt", name=f"xt_{b}_{t}")
        nc.gpsimd.dma_start(
            xt[:].rearrange("p (dt t) -> p dt t", dt=DT),
            xT_r[:, :, b * S + t * TC: b * S + (t + 1) * TC],
        )
        xts[(b, t)] = xt

    def load_w(w_sb, w_d):
        # [D, 128] DRAM -> [128, DT*128] SBUF, d-tile major, cast to bf16
        nc.gpsimd.dma_start(
            w_sb[:].rearrange("p (dt o) -> p dt o", dt=DT),
            w_d[:].rearrange("(dt p) o -> p dt o", p=P),
        )

    load_w(wk, wkT_d)
    load_x(0, 0)
    load_w(wq, wqT_d)
    load_x(0, 1)
    load_w(wv, wvT_d)
    nc.gpsimd.dma_start(wo[:], woT_d[:])

    # ---- per-batch tile registries (filled as the pipeline advances) ----
    qTs, kTs, vTs, vxs, outTs = {}, {}, {}, {}, {}
    exps = {}    # (b, qc, h, kt) -> exp tile [P, QC] bf16
    oes = {}     # (b, qc, h, half) -> attnv psum accumulator

    def get_batch_tiles(b):
        if b not in qTs:
            qTs[b] = qpool.tile([P, S], bf16, tag="qT", name=f"qT{b}")
            kTs[b] = kpool.tile([P, S], bf16, tag="kT", name=f"kT{b}")
            vTs[b] = vpool.tile([P, S], bf16, tag="vT", name=f"vT{b}")

    def proj(w_sb, dst_of, b, t):
        # one contiguous 8-matmul accumulation group + bf16 eviction
        get_batch_tiles(b)
        load_x(b, t + 2) if t + 2 < NTC else load_x(b + 1, t + 2 - NTC)
        pp = fpsum.tile([P, HQ], f32, tag="fp")
        for dt in range(DT):
            nc.tensor.matmul(
                pp[:],
                w_sb[:, dt * P:(dt + 1) * P],
                xts[(b, t)][:, dt * TC:(dt + 1) * TC],
                start=(dt == 0), stop=(dt == DT - 1),
            )
        nc.vector.tensor_copy(dst_of(b)[:, t * TC:(t + 1) * TC], pp[:])

    def get_vext(b):
        if b not in vxs:
            vx = vxpool.tile([P, KT * VB], bf16, tag="vext", name=f"vx{b}")
            # prewrite the ones columns (cols kt*VB + {64, 129}) once
            nc.vector.tensor_copy(
                vx[:].rearrange("p (kt ho c) -> p kt ho c", kt=KT, ho=2)
                    [:, :, :, HD:HD + 1],
                ones32[:].rearrange("p (kt ho c) -> p kt ho c", kt=KT, ho=2),
            )
            vxs[b] = vx
        return vxs[b]

    def vtrans(b, kt):
        vx = get_vext(b)
        vps = fpsum.tile([P, P], bf16, tag="fp")
        nc.tensor.transpose(vps[:], vTs[b][:, kt * P:(kt + 1) * P],
                            ident_b[:])
        base = kt * VB
        nc.vector.tensor_copy(vx[:, base:base + HD], vps[:, 0:HD])
        nc.vector.tensor_copy(vx[:, base + HD + 1:base + 2 * HD + 1],
                              vps[:, HD:2 * HD])

    def attnv(b, qc, h, half):
        # one contiguous 16-matmul accumulation group, then immediate
        # evacuation (numerator bf16 + denominator f32) to free the bank
        oe = apsum.tile([P, HQ], f32, tag="ap", name=f"oe_{b}_{qc}_{h}_{half}")
        vx = vxs[b]
        hq = slice(half * HQ, (half + 1) * HQ)
        for kt in range(KT):
            base = kt * VB + h * (HD + 1)
            nc.tensor.matmul(
                oe[0:HD + 1, :],
                vx[:, base:base + HD + 1],
                exps[(b, qc, h, kt)][:, hq],
                start=(kt == 0), stop=(kt == KT - 1),
            )
        num = rpool.tile([HD, HQ], bf16, tag="num")
        nc.vector.tensor_copy(num[:], oe[0:HD, :])
        den = rpool.tile([1, HQ], f32, tag="den")
        nc.vector.tensor_copy(den[:], oe[HD:HD + 1, :])
        oes[(b, qc, h, half)] = (num, den)

    def get_outT(b):
        if b not in outTs:
            outTs[b] = opool.tile([P, S], bf16, tag="outT", name=f"outT{b}")
        return outTs[b]

    def div(b, qc, h, half):
        num, den = oes.pop((b, qc, h, half))
        rec = rpool.tile([1, HQ], f32, tag="rec")
        scr = rpool.tile([1, HQ], f32, tag="scr")
        nc.vector.reciprocal_approx_accurate(rec[:], den[:], scr[:])
        recr = rpool.tile([1, HQ], f32r, tag="recr")
        with nc.allow_low_precision(reason="feeds bcast matmul"):
            nc.vector.tensor_copy(recr[:], rec[:])
        rb_ps = fpsum.tile([P, HQ], f32, tag="fp")
        nc.tensor.matmul(rb_ps[0:HD, :], ones_r[:], recr[:],
                         start=True, stop=True)
        rb = rpool.tile([HD, HQ], bf16, tag="rb")
        with nc.allow_low_precision(reason="softmax weights"):
            nc.vector.tensor_copy(rb[:], rb_ps[0:HD, :])
        qs = slice(qc * QC + half * HQ, qc * QC + (half + 1) * HQ)
        nc.vector.tensor_mul(get_outT(b)[h * HD:(h + 1) * HD, qs],
                             num[:], rb[:])

    def oproj(b, t):
        oT = outTs[b]
        osb = ospool.tile([P, D], bf16, tag="osb")
        for ec in range(D // HQ):
            po = fpsum.tile([P, HQ], f32, tag="fp")
            nc.tensor.matmul(
                po[:],
                oT[:, t * P:(t + 1) * P],
                wo[:, ec * HQ:(ec + 1) * HQ],
                start=True, stop=True,
            )
            nc.vector.tensor_copy(osb[:, ec * HQ:(ec + 1) * HQ], po[:])
        nc.sync.dma_start(
            out_d[b * S + t * P: b * S + (t + 1) * P, :],
            osb[:],
        )

    # ---------- work queue with deficit pacing + deadlines ----------
    queue = []   # list of [cost_cycles, deadline_slot, fn]
    state = {"deficit": 0.0}

    def push(cost, deadline, fn):
        queue.append([cost, deadline, fn])

    def pump(slot, allot):
        # force-emit past-deadline units (and, since queue order encodes
        # dependencies, everything queued before them), then paced emission
        state["deficit"] += allot
        last = -1
        for i, u in enumerate(queue):
            if u[1] <= slot:
                last = i
        for _ in range(last + 1):
            cost, _, fn = queue.pop(0)
            fn()
            state["deficit"] -= cost
        while queue and state["deficit"] > 0:
            cost, _, fn = queue.pop(0)
            fn()
            state["deficit"] -= cost

    NEVER = 10 ** 9
    C_PROJ, C_ATTNV, C_OPROJ, C_VT, C_DIV = 4096, 8192, 1024, 130, 512

    def slot_of(b, qc, kt):
        return (b * NQC + qc) * KT + kt

    def push_phase_supply(b, qc):
        """Queue the filler work for phase (b, qc), in dependency order."""
        base = slot_of(b, qc, 0)
        if qc == 0:
            pb, pqc = b - 1, 1
        else:
            pb, pqc = b, 0
        # attn@v + divisions of the previous phase's exps, then the
        # out-projections they unblock (qc0 -> t 0..7, qc1 -> t 8..15)
        if pb >= 0:
            t0 = 0 if pqc == 0 else 8
            push(C_ATTNV, NEVER, lambda: attnv(pb, pqc, 0, 0))
            push(C_ATTNV, NEVER, lambda: attnv(pb, pqc, 0, 1))
            push(C_DIV, NEVER, lambda: div(pb, pqc, 0, 0))
            push(C_ATTNV, NEVER, lambda: attnv(pb, pqc, 1, 0))
            push(C_DIV, NEVER, lambda: div(pb, pqc, 0, 1))
            push(C_ATTNV, NEVER, lambda: attnv(pb, pqc, 1, 1))
            push(C_DIV, NEVER, lambda: div(pb, pqc, 1, 0))
            for t_ in (t0, t0 + 1):
                push(C_OPROJ, NEVER, (lambda tt: lambda: oproj(pb, tt))(t_))
            push(C_DIV, NEVER, lambda: div(pb, pqc, 1, 1))
            for t_ in range(t0 + 2, t0 + 8):
                push(C_OPROJ, NEVER, (lambda tt: lambda: oproj(pb, tt))(t_))
        # next batch's projections + v transposes, split across the two
        # phases; deadlines pin them before their consumers.
        nb = b + 1
        if nb < B:
            dl = slot_of(nb, 0, 0)
            if qc == 0:
                # k0,q0,v0,k1,q1,v1 — k0/k1/q0/q1 needed by scores(nb, 0, 0)
                dls = {("k", 0): dl - 6, ("q", 0): dl - 5, ("v", 0): dl - 4,
                       ("k", 1): dl - 3, ("q", 1): dl - 2, ("v", 1): dl - 1}
                for t_ in (0, 1):
                    push(C_PROJ, dls[("k", t_)],
                         (lambda tt: lambda: proj(wk, kTs.get, nb, tt))(t_))
                    push(C_PROJ, dls[("q", t_)],
                         (lambda tt: lambda: proj(wq, qTs.get, nb, tt))(t_))
                    push(C_PROJ, dls[("v", t_)],
                         (lambda tt: lambda: proj(wv, vTs.get, nb, tt))(t_))
            else:
                # k2 needed at scores(nb,0,8), k3 at (nb,0,12),
                # q2/q3 at scores(nb,1,0), v2/v3 by the VT deadline
                dls = {("k", 2): dl + 6, ("q", 2): dl + 8, ("v", 2): dl + 10,
                       ("k", 3): dl + 10, ("q", 3): dl + 12, ("v", 3): dl + 12}
                for t_ in (2, 3):
                    push(C_PROJ, dls[("k", t_)],
                         (lambda tt: lambda: proj(wk, kTs.get, nb, tt))(t_))
                    push(C_PROJ, dls[("q", t_)],
                         (lambda tt: lambda: proj(wq, qTs.get, nb, tt))(t_))
                    push(C_PROJ, dls[("v", t_)],
                         (lambda tt: lambda: proj(wv, vTs.get, nb, tt))(t_))
                # v transposes of batch nb: consumed by attnv(nb, 0, ...)
                # which pops in phase (nb, 1)
                dl_vt = slot_of(nb, 1, 0)
                for kt0 in range(0, KT, 4):
                    def vt_bundle(bb, k0):
                        def f():
                            for k_ in range(k0, k0 + 4):
                                vtrans(bb, k_)
                        return f
                    push(4 * C_VT, dl_vt, vt_bundle(nb, kt0))

    def scores_kt(b, qc, kt):
        ps = {}
        for h in range(NH):
            ps[h] = spsum.tile([P, QC], f32, tag="sp",
                               name=f"sc_{b}_{qc}_{kt}_{h}")
        for half in range(NQC):
            hq = slice(half * HQ, (half + 1) * HQ)
            for h in range(NH):
                hs = slice(h * HD, (h + 1) * HD)
                nc.tensor.matmul(
                    ps[h][:, hq],
                    kTs[b][hs, kt * P:(kt + 1) * P],
                    qTs[b][hs, qc * QC + half * HQ:
                           qc * QC + (half + 1) * HQ],
                    start=True, stop=True,
                )
        for h in range(NH):
            ex = epool.tile([P, QC], bf16, tag="exp",
                            name=f"ex_{b}_{qc}_{h}_{kt}")
            nc.scalar.activation(ex[:], ps[h][:],
                                 mybir.ActivationFunctionType.Exp,
                                 scale=EXP_SCALE)
            exps[(b, qc, h, kt)] = ex

    # ---------- prologue: first projections for batch 0 ----------
    load_x(0, 2)
    proj(wk, kTs.get, 0, 0)
    proj(wq, qTs.get, 0, 0)
    proj(wq, qTs.get, 0, 1)

    # batch-0 remaining projections + v transposes, deadline-gated
    push(C_PROJ, slot_of(0, 0, 2), lambda: proj(wk, kTs.get, 0, 1))
    push(C_PROJ, slot_of(0, 0, 6), lambda: proj(wk, kTs.get, 0, 2))
    push(C_PROJ, slot_of(0, 0, 7), lambda: proj(wv, vTs.get, 0, 0))
    push(C_PROJ, slot_of(0, 0, 10), lambda: proj(wk, kTs.get, 0, 3))
    push(C_PROJ, slot_of(0, 0, 12), lambda: proj(wq, qTs.get, 0, 2))
    push(C_PROJ, slot_of(0, 0, 13), lambda: proj(wq, qTs.get, 0, 3))
    push(C_PROJ, slot_of(0, 0, 14), lambda: proj(wv, vTs.get, 0, 1))
    push(C_PROJ, slot_of(0, 1, 2), lambda: proj(wv, vTs.get, 0, 2))
    push(C_PROJ, slot_of(0, 1, 4), lambda: proj(wv, vTs.get, 0, 3))
    dl_vt0 = slot_of(0, 1, 15)
    for kt0 in range(0, KT, 4):
        def vt_bundle0(k0):
            def f():
                for k_ in range(k0, k0 + 4):
                    vtrans(0, k_)
            return f
        push(4 * C_VT, dl_vt0 - (3 - kt0 // 4), vt_bundle0(kt0))

    # ---------- main pipeline ----------
    total_slots = B * NQC * KT
    for b in range(B):
        for qc in range(NQC):
            push_phase_supply(b, qc)
            # phase allotment: spread queued cost over the 16 kt slots,
            # but cap so ScalarE stays the pacer (~5500 cyc/kt of ACT).
            pending = sum(u[0] for u in queue)
            allot = min(pending / KT, 5200.0)
            for kt in range(KT):
                scores_kt(b, qc, kt)
                pump(slot_of(b, qc, kt) + 1, allot)

    # ---------- epilogue: flush everything left ----------
    push_phase_supply(B, 0)   # attnv/div/oproj for (3, qc1)
    pump(NEVER + 1, 0)


def _build():
    global _CACHED_NC
    if _CACHED_NC is not None:
        return _CACHED_NC
    nc = bacc.Bacc("TRN2", target_bir_lowering=False, debug=False)
    xT = nc.dram_tensor("xT", [D, T], f32, kind="ExternalInput").ap()
    wqT = nc.dram_tensor("wqT", [D, P], f32, kind="ExternalInput").ap()
    wkT = nc.dram_tensor("wkT", [D, P], f32, kind="ExternalInput").ap()
    wvT = nc.dram_tensor("wvT", [D, P], f32, kind="ExternalInput").ap()
    woT = nc.dram_tensor("woT", [P, D], f32, kind="ExternalInput").ap()
    ones = nc.dram_tensor("ones", [P, 2 * KT + HD], f32,
                          kind="ExternalInput").ap()
    out = nc.dram_tensor("out", [T, D], bf16, kind="ExternalOutput").ap()

    with tile.TileContext(nc) as tc_:
        _mha_kernel(tc_, [xT, wqT, wkT, wvT, woT, ones], [out])
    nc.compile()
    _CACHED_NC = nc
    return nc


def kernel(x: np.ndarray, Wq: np.ndarray, Wk: np.ndarray, Wv: np.ndarray,
           Wo: np.ndarray) -> np.ndarray:
    global LAST_EXEC_TIME_NS
    nc = _build()

    x = np.asarray(x, dtype=np.float32)
    xT = np.ascontiguousarray(x.reshape(T, D).T)          # [D, T]
    ones_in = np.ones((P, 2 * KT + HD), dtype=np.float32)

    in_maps = []
    for c in range(NCORES):
        rows = slice(c * P, (c + 1) * P)
        in_maps.append({
            "xT": xT,
            "wqT": np.ascontiguousarray(np.asarray(Wq, np.float32)[rows, :].T),
            "wkT": np.ascontiguousarray(np.asarray(Wk, np.float32)[rows, :].T),
            "wvT": np.ascontiguousarray(np.asarray(Wv, np.float32)[rows, :].T),
            "woT": np.ascontiguousarray(np.asarray(Wo, np.float32)[:, rows].T),
            "ones": ones_in,
        })

    trace = bool(os.environ.get("BASS_TRACE"))
    res = run_bass_kernel_spmd(nc, in_maps, core_ids=list(range(NCORES)),
                               trace=trace)
    LAST_EXEC_TIME_NS = res.exec_time_ns

    acc = res.results[0]["out"].astype(np.float32)
    for c in range(1, NCORES):
        acc = acc + res.results[c]["out"].astype(np.float32)
    return acc.reshape(B, S, D)


# revision 30
# speedup vs baseline: 1.1884x; 1.0768x over previous
"""Multi-head attention (B=4, S=2048, D=1024, H=16) on 8 trn2 NeuronCores.

Sharding: tensor-parallel over heads — core c owns heads [2c, 2c+1]
(= model dims [128c, 128c+128)).  Each core computes q/k/v projections for
its head slice (full batch), local attention, and a partial out-projection
against its 128 columns of Wo.  The 8 partial [B*S, D] outputs (bf16) are
summed on the host (the all-reduce of the Megatron pattern, at gather time).

Per-core kernel (bf16 matmul operands, fp32 PSUM accumulation):
  xT  [D, B*S]   : x transposed on host so the contraction dim lands on
                   SBUF partitions (avoids on-chip transposes of x).
  qT/kT [128, S] per batch : head-major [2*64, tokens].
  scores_T [k, q] in PSUM, BOTH heads in one [128, 1024] 2-bank tile ->
                   ONE exp ACTIVATE (scale=1/8 fused) -> bf16.
  v_ext [k-tile, 2*(64+1)]: v natural layout (PE-transposed) with a ones
                   column per head => attn@v yields numerator (rows 0-63) +
                   softmax denominator (row 64) in one pass.
  division: denom row staged to partition 0, fast-NR reciprocal (fp32),
                   K=1 f32r matmul broadcast across partitions, DVE multiply.
  out-proj: outT [128, S] (head dims on partitions) @ WoT slice -> bf16 out.

Scheduling: the kernel is ScalarE-bound (33.5M exp elements/core ~ 290us of
ACT busy); PE total is ~290us.  Both must run >90% utilized, so the schedule
is one continuous software pipeline: 16 phases of 16 k-tiles each (q-chunk
QC=512 per phase).  Per k-tile, the two heads' score matmuls write one
shared PSUM tile and are freed by ONE ACT — both become WAR-ready at the
same instant, so Tile's readiness-driven scheduler issues them adjacently
and they run CONCURRENTLY on disjoint PE row groups (auto tile_position
from the kT/qT partition bases).  All other PE work (attn@v of the previous
q-chunk, divisions, out-projections, next batch's projections, v
transposes) flows from a deficit-paced FIFO work queue with per-unit
deadlines, filling the PE between score pairs.  PSUM: 3x 2-bank score
slots (3-deep so scores never wait on the exp of the previous k-tile) +
1 attn@v accumulator bank + 1 filler bank.  PSUM accumulation groups stay
contiguous in emission; the scheduler may interleave other-bank matmuls,
which is safe (has_written is per-bank).
"""
import os
import sys

sys.path.insert(0, "/opt/trn_rl_repo")

import numpy as np

import concourse.mybir as mybir
import concourse.tile as tile
from concourse import bacc
from concourse._compat import with_exitstack
from concourse.bass_utils import run_bass_kernel_spmd
from concourse.masks import make_identity
from contextlib import ExitStack

B, S, D, H = 4, 2048, 1024, 16
HD = D // H              # 64
P = 128
NCORES = 8
NH = H // NCORES         # 2 heads per core
T = B * S                # 8192 tokens
DT = D // P              # 8 contraction tiles
KT = S // P              # 16 k-tiles per batch
QC = 512                 # q-chunk = one psum bank per head
NQC = S // QC            # 4 phases per batch
TC = 512                 # projection token chunk
NTC = S // TC            # 4
VB = 2 * (HD + 1)        # v_ext block per k-tile: [v0|1|v1|1]
EXP_SCALE = float(1.0 / np.sqrt(HD))

f32 = mybir.dt.float32
f32r = mybir.dt.float32r
bf16 = mybir.dt.bfloat16

LAST_EXEC_TIME_NS = None
_CACHED_NC = None


@with_exitstack
def _mha_kernel(ctx: ExitStack, tc_: tile.TileContext, ins, outs):
    nc = tc_.nc
    xT_d, wqT_d, wkT_d, wvT_d, woT_d, ones_d = ins
    out_d = outs[0]

    const = ctx.enter_context(tc_.tile_pool(name="const", bufs=1))
    wpool = ctx.enter_context(tc_.tile_pool(name="wpool", bufs=1))
    xpool = ctx.enter_context(tc_.tile_pool(name="xpool", bufs=3))
    qpool = ctx.enter_context(tc_.tile_pool(name="qpool", bufs=2))
    kpool = ctx.enter_context(tc_.tile_pool(name="kpool", bufs=2))
    vpool = ctx.enter_context(tc_.tile_pool(name="vpool", bufs=2))
    vxpool = ctx.enter_context(tc_.tile_pool(name="vxpool", bufs=3))
    epool = ctx.enter_context(tc_.tile_pool(name="epool", bufs=40))
    opool = ctx.enter_context(tc_.tile_pool(name="opool", bufs=2))
    rpool = ctx.enter_context(tc_.tile_pool(name="rpool", bufs=2))
    ospool = ctx.enter_context(tc_.tile_pool(name="ospool", bufs=2))

    # PSUM: 3x 2-bank score slots + 1 attnv accumulator + 1 filler = 8 banks
    spsum = ctx.enter_context(tc_.tile_pool(name="spsum", bufs=3, space="PSUM"))
    apsum = ctx.enter_context(tc_.tile_pool(name="apsum", bufs=1, space="PSUM"))
    fpsum = ctx.enter_context(tc_.tile_pool(name="fpsum", bufs=1, space="PSUM"))

    ident = const.tile([P, P], f32, tag="ident")
    make_identity(nc, ident)
    ident_b = const.tile([P, P], bf16, tag="ident_b")
    nc.vector.tensor_copy(ident_b[:], ident[:])
    ones32 = const.tile([P, 2 * KT], bf16, tag="ones32")
    nc.gpsimd.dma_start(ones32[:], ones_d[:, 0:2 * KT])
    ones_r = const.tile([1, HD], f32r, tag="ones_r")
    nc.sync.dma_start(ones_r[:], ones_d[0:1, 2 * KT:2 * KT + HD].bitcast(f32r))

    wq = wpool.tile([P, D], bf16, tag="wq")
    wk = wpool.tile([P, D], bf16, tag="wk")
    wv = wpool.tile([P, D], bf16, tag="wv")
    wo = wpool.tile([P, D], bf16, tag="wo")

    xT_r = xT_d[:].rearrange("(dt p) t -> p dt t", p=P)

    xts = {}

    def load_x(b, t):
        if (b, t) in xts or b >= B:
            return
        xt = xpool.tile([P, DT * TC], bf16, tag="xt", name=f"xt_{b}_{t}")
        nc.gpsimd.dma_start(
            xt[:].rearrange("p (dt t) -> p dt t", dt=DT),
            xT_r[:, :, b * S + t * TC: b * S + (t + 1) * TC],
        )
        xts[(b, t)] = xt

    def load_w(w_sb, w_d):
        # [D, 128] DRAM -> [128, DT*128] SBUF, d-tile major, cast to bf16
        nc.gpsimd.dma_start(
            w_sb[:].rearrange("p (dt o) -> p dt o", dt=DT),
            w_d[:].rearrange("(dt p) o -> p dt o", p=P),
        )

    load_w(wk, wkT_d)
    load_x(0, 0)
    load_w(wq, wqT_d)
    load_x(0, 1)
    load_w(wv, wvT_d)
    nc.gpsimd.dma_start(wo[:], woT_d[:])

    # ---- per-batch tile registries (filled as the pipeline advances) ----
    qTs, kTs, vTs, vxs, outTs = {}, {}, {}, {}, {}
    exps = {}    # (b, qc, kt) -> exp tile [P, 2*QC] bf16 (h0|h1)
    oes = {}     # (b, qc, h) -> (numer bf16 sbuf, denom f32 sbuf)

    def get_batch_tiles(b):
        if b not in qTs:
            qTs[b] = qpool.tile([P, S], bf16, tag="qT", name=f"qT{b}")
            kTs[b] = kpool.tile([P, S], bf16, tag="kT", name=f"kT{b}")
            vTs[b] = vpool.tile([P, S], bf16, tag="vT", name=f"vT{b}")

    def proj(w_sb, dst_of, b, t):
        # one contiguous 8-matmul accumulation group + bf16 eviction
        get_batch_tiles(b)
        load_x(b, t + 2) if t + 2 < NTC else load_x(b + 1, t + 2 - NTC)
        pp = fpsum.tile([P, TC], f32, tag="fp")
        for dt in range(DT):
            nc.tensor.matmul(
                pp[:],
                w_sb[:, dt * P:(dt + 1) * P],
                xts[(b, t)][:, dt * TC:(dt + 1) * TC],
                start=(dt == 0), stop=(dt == DT - 1),
            )
        nc.vector.tensor_copy(dst_of(b)[:, t * TC:(t + 1) * TC], pp[:])

    def get_vext(b):
        if b not in vxs:
            vx = vxpool.tile([P, KT * VB], bf16, tag="vext", name=f"vx{b}")
            # prewrite the ones columns (cols kt*VB + {64, 129}) once
            nc.vector.tensor_copy(
                vx[:].rearrange("p (kt ho c) -> p kt ho c", kt=KT, ho=2)
                    [:, :, :, HD:HD + 1],
                ones32[:].rearrange("p (kt ho c) -> p kt ho c", kt=KT, ho=2),
            )
            vxs[b] = vx
        return vxs[b]

    def vtrans(b, kt):
        vx = get_vext(b)
        vps = fpsum.tile([P, P], bf16, tag="fp")
        nc.tensor.transpose(vps[:], vTs[b][:, kt * P:(kt + 1) * P],
                            ident_b[:])
        base = kt * VB
        nc.vector.tensor_copy(vx[:, base:base + HD], vps[:, 0:HD])
        nc.vector.tensor_copy(vx[:, base + HD + 1:base + 2 * HD + 1],
                              vps[:, HD:2 * HD])

    def attnv(b, qc, h):
        # one contiguous 16-matmul accumulation group, then immediate
        # evacuation (numerator bf16 + denominator f32) to free the bank
        oe = apsum.tile([P, QC], f32, tag="ap", name=f"oe_{b}_{qc}_{h}")
        vx = vxs[b]
        hq = slice(h * QC, (h + 1) * QC)
        for kt in range(KT):
            base = kt * VB + h * (HD + 1)
            nc.tensor.matmul(
                oe[0:HD + 1, :],
                vx[:, base:base + HD + 1],
                exps[(b, qc, kt)][:, hq],
                start=(kt == 0), stop=(kt == KT - 1),
            )
        num = rpool.tile([HD, QC], bf16, tag="num")
        nc.vector.tensor_copy(num[:], oe[0:HD, :])
        den = rpool.tile([1, QC], f32, tag="den")
        nc.vector.tensor_copy(den[:], oe[HD:HD + 1, :])
        oes[(b, qc, h)] = (num, den)

    def get_outT(b):
        if b not in outTs:
            outTs[b] = opool.tile([P, S], bf16, tag="outT", name=f"outT{b}")
        return outTs[b]

    def div(b, qc, h):
        num, den = oes.pop((b, qc, h))
        rec = rpool.tile([1, QC], f32, tag="rec")
        scr = rpool.tile([1, QC], f32, tag="scr")
        nc.vector.reciprocal_approx_accurate(rec[:], den[:], scr[:])
        recr = rpool.tile([1, QC], f32r, tag="recr")
        with nc.allow_low_precision(reason="feeds bcast matmul"):
            nc.vector.tensor_copy(recr[:], rec[:])
        rb_ps = fpsum.tile([P, QC], f32, tag="fp")
        nc.tensor.matmul(rb_ps[0:HD, :], ones_r[:], recr[:],
                         start=True, stop=True)
        rb = rpool.tile([HD, QC], bf16, tag="rb")
        with nc.allow_low_precision(reason="softmax weights"):
            nc.vector.tensor_copy(rb[:], rb_ps[0:HD, :])
        qs = slice(qc * QC, (qc + 1) * QC)
        nc.vector.tensor_mul(get_outT(b)[h * HD:(h + 1) * HD, qs],
                             num[:], rb[:])

    def oproj(b, t):
        oT = outTs[b]
        osb = ospool.tile([P, D], bf16, tag="osb")
        for ec in range(D // TC):
            po = fpsum.tile([P, TC], f32, tag="fp")
            nc.tensor.matmul(
                po[:],
                oT[:, t * P:(t + 1) * P],
                wo[:, ec * TC:(ec + 1) * TC],
                start=True, stop=True,
            )
            nc.vector.tensor_copy(osb[:, ec * TC:(ec + 1) * TC], po[:])
        nc.sync.dma_start(
            out_d[b * S + t * P: b * S + (t + 1) * P, :],
            osb[:],
        )

    # ---------- work queue with deficit pacing + deadlines ----------
    queue = []   # list of [cost_cycles, deadline_slot, fn]
    state = {"deficit": 0.0}

    def push(cost, deadline, fn):
        queue.append([cost, deadline, fn])

    def pump(slot, allot):
        # force-emit past-deadline units (and, since queue order encodes
        # dependencies, everything queued before them), then paced emission
        state["deficit"] += allot
        last = -1
        for i, u in enumerate(queue):
            if u[1] <= slot:
                last = i
        for _ in range(last + 1):
            cost, _, fn = queue.pop(0)
            fn()
            state["deficit"] -= cost
        while queue and state["deficit"] > 0:
            cost, _, fn = queue.pop(0)
            fn()
            state["deficit"] -= cost

    NEVER = 10 ** 9
    C_PROJ, C_ATTNV, C_OPROJ, C_VT, C_DIV = 4096, 8192, 1024, 130, 512

    def slot_of(b, qc, kt):
        return (b * NQC + qc) * KT + kt

    def push_phase_supply(b, qc):
        """Queue the filler work for phase (b, qc), in dependency order."""
        if qc == 0:
            pb, pqc = b - 1, NQC - 1
        else:
            pb, pqc = b, qc - 1
        # attn@v + divisions over the previous phase's exps, then the
        # out-projections they unblock (t-tiles 4*pqc .. 4*pqc+3)
        if pb >= 0:
            push(C_ATTNV, NEVER, lambda: attnv(pb, pqc, 0))
            push(C_ATTNV, NEVER, lambda: attnv(pb, pqc, 1))
            push(C_DIV, NEVER, lambda: div(pb, pqc, 0))
            push(C_DIV, NEVER, lambda: div(pb, pqc, 1))
            for t_ in range(4 * pqc, 4 * pqc + 4):
                push(C_OPROJ, NEVER, (lambda tt: lambda: oproj(pb, tt))(t_))
        # next batch's projections (chunk qc) + one v-transpose bundle
        nb = b + 1
        if nb < B:
            c = qc
            dl0 = slot_of(nb, 0, 0)
            push(C_PROJ, dl0 + 4 * c - 2,
                 (lambda tt: lambda: proj(wk, kTs.get, nb, tt))(c))
            push(C_PROJ, max(slot_of(nb, c, 0) - 2, dl0 - 2),
                 (lambda tt: lambda: proj(wq, qTs.get, nb, tt))(c))
            push(C_PROJ, slot_of(nb, 1, 0) - 8,
                 (lambda tt: lambda: proj(wv, vTs.get, nb, tt))(c))
            dl_vt = slot_of(nb, 1, 0) - 2

            def vt_bundle(bb, k0):
                def f():
                    for k_ in range(k0, k0 + 4):
                        vtrans(bb, k_)
                return f
            push(4 * C_VT, dl_vt, vt_bundle(nb, 4 * c))

    def scores_kt(b, qc, kt):
        # both heads' scores in ONE 2-bank tile; the two matmuls are
        # row-group concurrent; one ACT over both frees the slot at once
        ps = spsum.tile([P, 2 * QC], f32, tag="sp",
                        name=f"sc_{b}_{qc}_{kt}")
        for h in range(NH):
            hs = slice(h * HD, (h + 1) * HD)
            nc.tensor.matmul(
                ps[:, h * QC:(h + 1) * QC],
                kTs[b][hs, kt * P:(kt + 1) * P],
                qTs[b][hs, qc * QC:(qc + 1) * QC],
                start=True, stop=True,
            )
        ex = epool.tile([P, 2 * QC], bf16, tag="exp",
                        name=f"ex_{b}_{qc}_{kt}")
        nc.scalar.activation(ex[:], ps[:],
                             mybir.ActivationFunctionType.Exp,
                             scale=EXP_SCALE)
        exps[(b, qc, kt)] = ex

    # ---------- prologue: first projections for batch 0 ----------
    load_x(0, 2)
    proj(wk, kTs.get, 0, 0)
    proj(wq, qTs.get, 0, 0)

    # batch-0 remaining projections + v transposes, deadline-gated
    def vt_bundle0(k0):
        def f():
            for k_ in range(k0, k0 + 4):
                vtrans(0, k_)
        return f
    push(C_PROJ, slot_of(0, 0, 2), lambda: proj(wk, kTs.get, 0, 1))
    push(C_PROJ, slot_of(0, 0, 4), lambda: proj(wv, vTs.get, 0, 0))
    push(C_PROJ, slot_of(0, 0, 6), lambda: proj(wk, kTs.get, 0, 2))
    push(C_PROJ, slot_of(0, 0, 8), lambda: proj(wv, vTs.get, 0, 1))
    push(C_PROJ, slot_of(0, 0, 10), lambda: proj(wk, kTs.get, 0, 3))
    push(4 * C_VT, slot_of(0, 0, 12), vt_bundle0(0))
    push(4 * C_VT, slot_of(0, 0, 13), vt_bundle0(4))
    push(C_PROJ, slot_of(0, 1, 0) - 2, lambda: proj(wq, qTs.get, 0, 1))
    push(C_PROJ, slot_of(0, 1, 0) - 1, lambda: proj(wv, vTs.get, 0, 2))
    push(4 * C_VT, slot_of(0, 1, 0), vt_bundle0(8))
    push(C_PROJ, slot_of(0, 1, 2), lambda: proj(wv, vTs.get, 0, 3))
    push(4 * C_VT, slot_of(0, 1, 4), vt_bundle0(12))
    push(C_PROJ, slot_of(0, 2, 0) - 2, lambda: proj(wq, qTs.get, 0, 2))
    push(C_PROJ, slot_of(0, 3, 0) - 2, lambda: proj(wq, qTs.get, 0, 3))

    # ---------- main pipeline ----------
    for b in range(B):
        for qc in range(NQC):
            push_phase_supply(b, qc)
            pending = sum(u[0] for u in queue)
            allot = min(pending / KT, 5200.0)
            for kt in range(KT):
                scores_kt(b, qc, kt)
                pump(slot_of(b, qc, kt) + 1, allot)

    # ---------- epilogue: flush everything left ----------
    push_phase_supply(B, 0)   # attnv/div/oproj for (3, 3)
    pump(NEVER + 1, 0)


def _build():
    global _CACHED_NC
    if _CACHED_NC is not None:
        return _CACHED_NC
    nc = bacc.Bacc("TRN2", target_bir_lowering=False, debug=False)
    xT = nc.dram_tensor("xT", [D, T], f32, kind="ExternalInput").ap()
    wqT = nc.dram_tensor("wqT", [D, P], f32, kind="ExternalInput").ap()
    wkT = nc.dram_tensor("wkT", [D, P], f32, kind="ExternalInput").ap()
    wvT = nc.dram_tensor("wvT", [D, P], f32, kind="ExternalInput").ap()
    woT = nc.dram_tensor("woT", [P, D], f32, kind="ExternalInput").ap()
    ones = nc.dram_tensor("ones", [P, 2 * KT + HD], f32,
                          kind="ExternalInput").ap()
    out = nc.dram_tensor("out", [T, D], bf16, kind="ExternalOutput").ap()

    with tile.TileContext(nc) as tc_:
        _mha_kernel(tc_, [xT, wqT, wkT, wvT, woT, ones], [out])
    nc.compile()
    _CACHED_NC = nc
    return nc


def kernel(x: np.ndarray, Wq: np.ndarray, Wk: np.ndarray, Wv: np.ndarray,
           Wo: np.ndarray) -> np.ndarray:
    global LAST_EXEC_TIME_NS
    nc = _build()

    x = np.asarray(x, dtype=np.float32)
    xT = np.ascontiguousarray(x.reshape(T, D).T)          # [D, T]
    ones_in = np.ones((P, 2 * KT + HD), dtype=np.float32)

    in_maps = []
    for c in range(NCORES):
        rows = slice(c * P, (c + 1) * P)
        in_maps.append({
            "xT": xT,
            "wqT": np.ascontiguousarray(np.asarray(Wq, np.float32)[rows, :].T),
            "wkT": np.ascontiguousarray(np.asarray(Wk, np.float32)[rows, :].T),
            "wvT": np.ascontiguousarray(np.asarray(Wv, np.float32)[rows, :].T),
            "woT": np.ascontiguousarray(np.asarray(Wo, np.float32)[:, rows].T),
            "ones": ones_in,
        })

    trace = bool(os.environ.get("BASS_TRACE"))
    res = run_bass_kernel_spmd(nc, in_maps, core_ids=list(range(NCORES)),
                               trace=trace)
    LAST_EXEC_TIME_NS = res.exec_time_ns

    acc = res.results[0]["out"].astype(np.float32)
    for c in range(1, NCORES):
        acc = acc + res.results[c]["out"].astype(np.float32)
    return acc.reshape(B, S, D)
